# revision 20
# baseline (speedup 1.0000x reference)
"""Masked self-attention Trainium2 Bass kernel (fp8 DoubleRow edition).

Reference computation (per batch b):
    q = x @ Wq + bq ; k = x @ Wk + bk ; v = x @ Wv + bv      # [S, A]
    scores = (q @ k.T) / sqrt(S)  with causal mask            # [S, S]
    out = softmax(scores, axis=-1) @ v                        # [S, A]

Sharding: data-parallel over batch across 8 NeuronCores (B=32 -> 4 per core),
weights replicated. No collectives.

Per-core design. The PE is the bottleneck (92% busy in the fp32r version),
so the big matmuls run in fp8e4 (e4m3) with MatmulPerfMode.DoubleRow: two
128-row contraction subtiles packed as [128, 2, free] APs, 2x the fp32r/bf16
rate. Error budget (gate 2e-2, fp32r baseline 2.4e-4): fp8 weights/acts give
~5% per-element error on q/k/v. That is fine wherever softmax averages many
keys, but output row q attends q+1 keys, so rows 0..127 (whose only key tile
is tile 0) need an accurate path. numpy emulation of exactly this split
measures 6.7e-3 max-rel error.

  fp8 main path (all of S, used by output rows >= 128):
    xT8 [E,S] fp8  <- PE-transpose of DMA-fed x (fp32r, 1.5cyc/row), DVE
                      PSUM->SBUF copy quantizes.
    qT8/kT8 [A,S] fp8 <- DoubleRow proj (W8 pairs x xT8 pairs), ACT copy
                      adds bias, quantizes. Unscaled (1/sqrt(S) folds into
                      the exp activation's scale to keep fp8 values in the
                      normal range).
    v8 [S,A+2] fp8 <- DoubleRow proj for s-tiles 1..7; tile 0 quantized from
                      the accurate bf16 tile-0 PSUM. Two ones-columns give
                      softmax row sums via the PV matmul.
    scoresT per k-tile = DoubleRow(kT8 pair, qT8 pair) -> fp32 PSUM; -1e9
                      additive mask on diagonal blocks; ACT exp(scale*x) ->
                      expT8 fp8. Tail rows of partial tile 7 are zeroed so
                      DoubleRow pairs never read garbage.
    PV per q-tile i>=1: DoubleRow over k-tile pairs (+ single fp8 matmul for
                      odd leftover tile), accumulating [il, 256]+[il, 258]
                      PSUM; sum column -> DVE reciprocal -> scale; GPSIMD
                      adds bv; DMA out per 256-col half.
  accurate tile-0 path (output rows 0..127):
    xT0b [E,128] bf16 (extra copy of the s-tile-0 transposes), bf16 weights.
    qT0b/kT0b [A,128] bf16 proj; diag scores bf16; exp -> e0b bf16;
    v0b [128,A+2] bf16; PV0 = (e0b, v0b) bf16.
"""

import numpy as np
from contextlib import ExitStack

import concourse.bass as bass
import concourse.mybir as mybir
import concourse.tile as tile
from concourse import bacc
from concourse.bass_utils import run_bass_kernel_spmd
from concourse.masks import make_identity

P = 128
F32 = mybir.dt.float32
F32R = mybir.dt.float32r
F8 = mybir.dt.float8e4
BF = mybir.dt.bfloat16
AF = mybir.ActivationFunctionType
DR = mybir.MatmulPerfMode.DoubleRow

N_CORES = 8
B, S, E, A = 32, 1000, 1024, 512
MASK_NEG = -1.0e9
RB = 64  # rows handled by the accurate bf16 path (error ~ 1/sqrt(n_keys))


def _even_chunks(start, total, maxc):
    """Split [start, start+total) into ceil(total/maxc) near-even chunks,
    each of even size."""
    assert total % 2 == 0
    n = max(1, -(-total // maxc))
    bounds = [start + ((i * total) // n) // 2 * 2 for i in range(n)]
    bounds.append(start + total)
    return [(bounds[i], bounds[i + 1] - bounds[i]) for i in range(n)]


def build(b_pc, s, e, a, reps=1):
    assert e % P == 0 and a % P == 0
    n_s = -(-s // P)
    n_e = e // P
    n_a = a // P
    inv_den = float(s) ** -0.5
    s_tiles = [(t * P, min(P, s - t * P)) for t in range(n_s)]
    h = a // 2  # PV column split: [0,h) and [h, a+2)
    # DoubleRow pair strides (dim1 of a [P, n, cols] tile) must be multiples
    # of 64 bytes, so pad the inner dim of every pair-sliced fp8 tile
    s_pad = -(-s // 64) * 64
    v_pad = -(-(a + 2) // 64) * 64

    nc = bacc.Bacc("TRN2")
    x = nc.dram_tensor("x", [b_pc, s, e], F32, kind="ExternalInput").ap()
    w_dram = {
        "q": nc.dram_tensor("Wq", [e, a], F32, kind="ExternalInput").ap(),
        "k": nc.dram_tensor("Wk", [e, a], F32, kind="ExternalInput").ap(),
        "v": nc.dram_tensor("Wv", [e, a], F32, kind="ExternalInput").ap(),
    }
    b_dram = {
        "q": nc.dram_tensor("bq", [a], F32, kind="ExternalInput").ap(),
        "k": nc.dram_tensor("bk", [a], F32, kind="ExternalInput").ap(),
        "v": nc.dram_tensor("bv", [a], F32, kind="ExternalInput").ap(),
    }
    out = nc.dram_tensor("out", [b_pc, s, a], F32, kind="ExternalOutput").ap()

    with tile.TileContext(nc) as tc, ExitStack() as ctx:
        pool = ctx.enter_context(tc.tile_pool(name="sb", bufs=1))
        pp_tp = ctx.enter_context(tc.tile_pool(name="pp_tp", bufs=2, space="PSUM"))
        pp_proj = ctx.enter_context(tc.tile_pool(name="pp_proj", bufs=2, space="PSUM"))
        pp_score = ctx.enter_context(tc.tile_pool(name="pp_sc", bufs=2, space="PSUM"))
        pp_o1 = ctx.enter_context(tc.tile_pool(name="pp_o1", bufs=1, space="PSUM"))
        pp_o2 = ctx.enter_context(tc.tile_pool(name="pp_o2", bufs=1, space="PSUM"))

        # ---------------- constants ----------------
        ident_st = pool.tile([P, P], F32)
        make_identity(nc, ident_st)
        ident = pool.tile([P, P], BF)
        nc.scalar.copy(ident[:], ident_st[:])

        # ---------------- weights / biases ----------------
        # w8: [P, n_e, a] fp8 (DoubleRow pair-sliceable); wb: bf16 (tile-0)
        w8, wb = {}, {}
        for nm in ("q", "k", "v"):
            w8[nm] = pool.tile([P, n_e, a], F8, tag=f"w8_{nm}", bufs=1,
                               name=f"w8_{nm}")
            wb[nm] = pool.tile([P, n_e, a], BF, tag=f"wb_{nm}", bufs=1,
                               name=f"wb_{nm}")
            for u in range(n_e):
                w_stage = pool.tile([P, a], F32, tag="w_stage", bufs=2)
                nc.gpsimd.dma_start(w_stage[:], w_dram[nm][u * P:(u + 1) * P, :])
                nc.vector.tensor_copy(w8[nm][:, u], w_stage[:])
                nc.vector.tensor_copy(wb[nm][:, u], w_stage[:])

        bias_sb = {}
        for nm in ("q", "k"):
            b_st = pool.tile([P, n_a], F32, tag=f"b_{nm}", bufs=1)
            nc.gpsimd.dma_start(
                b_st[:], b_dram[nm].rearrange("(m p) -> p m", p=P)
            )
            bias_sb[nm] = b_st

        bv_stage = pool.tile([1, a], F32)
        nc.gpsimd.dma_start(bv_stage[:], b_dram["v"][:])
        bv_bc = pool.tile([P, a], F32)
        nc.gpsimd.partition_broadcast(bv_bc[:], bv_stage[:])

        # ---------------- per-batch pipeline ----------------
        rep_ctx = (tc.For_i(0, reps, 1, hint_engines=tuple(nc.engines),
                            staggered_reset=True)
                   if reps > 1 else None)
        if rep_ctx is not None:
            ctx.enter_context(rep_ctx)
        for b in range(b_pc):
            # ---- tiles for this batch ----
            xT8 = pool.tile([P, n_e, s_pad], F8, tag="xT8", bufs=2,
                            name=f"xT8_{b}")
            xT0b = pool.tile([P, n_e, P], BF, tag="xT0b", bufs=2,
                             name=f"xT0b_{b}")
            qkT8 = {}
            for nm in ("q", "k"):
                qkT8[nm] = pool.tile([P, n_a, s_pad], F8, tag=f"{nm}T8",
                                     bufs=2, name=f"{nm}T8_{b}")
            qkT0b = {}
            for nm in ("q", "k"):
                qkT0b[nm] = pool.tile([P, n_a, RB], BF, tag=f"{nm}T0b",
                                      bufs=2, name=f"{nm}T0b_{b}")
            v8 = pool.tile([P, n_s, v_pad], F8, tag="v8", bufs=2,
                           name=f"v8_{b}")
            v0b = pool.tile([P, a + 2], BF, tag="v0b", bufs=2, name=f"v0b_{b}")
            g = 4 if n_e % 4 == 0 else (2 if n_e % 2 == 0 else 1)

            # ones columns (softmax row sums) + zeroed tail rows of partial
            # tile 7 (so DoubleRow pair reads see finite zeros, not junk);
            # valid rows/cols are overwritten by the PSUM copies below
            sl_last = s_tiles[-1][1]
            if sl_last < P:
                z0 = (sl_last // 32) * 32
                nc.gpsimd.memset(v8[z0:, n_s - 1, :a], 0.0)
            nc.gpsimd.memset(v8[:, :, a:a + 2], 1.0)
            nc.gpsimd.memset(v0b[:, a:a + 2], 1.0)

            def load_and_transpose(t):
                s0, sl = s_tiles[t]
                x_sb = pool.tile([P, e], F32, tag="x", bufs=3, name="x_sb")
                xb_sb = pool.tile([P, e], BF, tag="xb", bufs=3, name="xb_sb")
                # finer DMA split for batch 0, whose loads pace pipeline fill
                nsp = 4 if b == 0 else 2
                w_sp = e // nsp
                for qi in range(nsp):
                    nc.sync.dma_start(
                        x_sb[:sl, qi * w_sp:(qi + 1) * w_sp],
                        x[b, s0:s0 + sl, qi * w_sp:(qi + 1) * w_sp],
                    )
                    # fp32 -> bf16 on GPSIMD (has slack); bf16 transposes run
                    # at 1 cyc/row on the PE vs 1.5 for fp32r
                    nc.gpsimd.tensor_copy(
                        xb_sb[:sl, qi * w_sp:(qi + 1) * w_sp],
                        x_sb[:sl, qi * w_sp:(qi + 1) * w_sp],
                    )
                for u0 in range(0, n_e, g):
                    tp = pp_tp.tile([P, g * P], BF, tag="tp", name="tp")
                    for j in range(g):
                        nc.tensor.transpose(
                            tp[:, j * P:j * P + sl],
                            xb_sb[:sl, (u0 + j) * P:(u0 + j + 1) * P],
                            ident[:sl, :sl],
                        )
                    tp3 = tp.rearrange("p (j c) -> p j c", c=P)
                    # alternate PSUM->SBUF copies across DVE/ACT to balance
                    if (t + u0 // g) % 2 == 0:
                        nc.vector.tensor_copy(
                            xT8[:, u0:u0 + g, s0:s0 + sl], tp3[:, :, :sl]
                        )
                    else:
                        nc.scalar.copy(
                            xT8[:, u0:u0 + g, s0:s0 + sl], tp3[:, :, :sl]
                        )
                    if s0 == 0:
                        nc.vector.tensor_copy(
                            xT0b[:, u0:u0 + g, :], tp3[:, :, :]
                        )

            def qk_chunk(nm, m, c0, cl):
                # fp8 qT/kT [A, S], unscaled, bias added in the ACT copy
                mm = pp_proj.tile([P, 512], F32, tag="proj", name="mm")
                for j in range(n_e // 2):
                    nc.tensor.matmul(
                        mm[:, :cl],
                        w8[nm][:, 2 * j:2 * j + 2, m * P:(m + 1) * P],
                        xT8[:, 2 * j:2 * j + 2, c0:c0 + cl],
                        start=(j == 0), stop=(j == n_e // 2 - 1),
                        perf_mode=DR,
                    )
                nc.scalar.activation(
                    qkT8[nm][:, m, c0:c0 + cl], mm[:, :cl], AF.Identity,
                    bias=bias_sb[nm][:, m:m + 1],
                )

            def v_tile(t):
                # fp8 DoubleRow proj; bv folded into v (softmax rows sum to
                # 1, so adding bv to every v row adds it to the output)
                s0, sl = s_tiles[t]
                vm = pp_proj.tile([P, 512], F32, tag="proj", name="vm")
                for j in range(n_e // 2):
                    nc.tensor.matmul(
                        vm[:sl, :a],
                        xT8[:, 2 * j:2 * j + 2, s0:s0 + sl],
                        w8["v"][:, 2 * j:2 * j + 2, :],
                        start=(j == 0), stop=(j == n_e // 2 - 1),
                        perf_mode=DR,
                    )
                nc.vector.tensor_add(v8[:sl, t, :a], vm[:sl, :a],
                                     bv_bc[:sl, :])

            # ---- stage A/B interleaved: first half of s, early proj ----
            for t in range(n_s // 2):
                load_and_transpose(t)

            # bf16 tile-0 qT/kT [A, RB] (accurate path for rows 0..RB-1)
            for nm in ("q", "k"):
                mm0 = pp_proj.tile([P, 512], F32, tag="proj", name="mm0")
                for m in range(n_a):
                    for u in range(n_e):
                        nc.tensor.matmul(
                            mm0[:, m * RB:(m + 1) * RB],
                            wb[nm][:, u, m * P:(m + 1) * P],
                            xT0b[:, u, :RB],
                            start=(u == 0), stop=(u == n_e - 1),
                        )
                for m in range(n_a):
                    nc.scalar.activation(
                        qkT0b[nm][:, m, :], mm0[:, m * RB:(m + 1) * RB],
                        AF.Identity, bias=bias_sb[nm][:, m:m + 1],
                    )

            # v tile 0: bf16 accurate proj -> both v0b and v8[:,0]
            vm0 = pp_proj.tile([P, 512], F32, tag="proj", name="vm0")
            for u in range(n_e):
                nc.tensor.matmul(
                    vm0[:, :a], xT0b[:, u, :], wb["v"][:, u, :],
                    start=(u == 0), stop=(u == n_e - 1),
                )
            nc.vector.tensor_add(v0b[:, :a], vm0[:, :a], bv_bc[:, :])
            nc.vector.tensor_add(v8[:, 0, :a], vm0[:, :a], bv_bc[:, :])

            half = (n_s // 2) * P  # 512: columns covered by tiles 0..3
            for nm in ("q", "k"):
                for m in range(n_a):
                    qk_chunk(nm, m, 0, half)
            for t in range(1, n_s // 2):
                v_tile(t)

            # ---- second half of s ----
            for t in range(n_s // 2, n_s):
                load_and_transpose(t)
            for nm in ("q", "k"):
                for m in range(n_a):
                    qk_chunk(nm, m, half, s - half)
            for t in range(n_s // 2, n_s):
                v_tile(t)

            # ---- stages C+D interleaved per tile ----
            expT8 = pool.tile([P, n_s, s_pad], F8, tag="expT8", bufs=2,
                              name=f"expT8_{b}")
            e0b = pool.tile([P, RB], BF, tag="e0b", bufs=2, name=f"e0b_{b}")

            for i, (q0, il) in enumerate(s_tiles):
                t, (k0, kl) = i, s_tiles[i]
                # --- scores + exp for k-tile t ---
                if t == 0:
                    # bf16 diagonal block (output rows 0..RB-1)
                    sc0 = pp_score.tile([P, 512], F32, tag="score")
                    for m in range(n_a):
                        nc.tensor.matmul(
                            sc0[:RB, :RB],
                            qkT0b["k"][:, m, :], qkT0b["q"][:, m, :],
                            start=(m == 0), stop=(m == n_a - 1),
                        )
                    nc.scalar.activation(e0b[:RB, :], sc0[:RB, :RB], AF.Exp,
                                         scale=inv_den)
                    # causal mask: zero exp where col q < row k (on GPSIMD,
                    # keeping DVE off the critical PSUM path; unmasked
                    # scores stay small enough that exp cannot overflow)
                    nc.gpsimd.affine_select(
                        out=e0b[:RB, :], in_=e0b[:RB, :],
                        compare_op=mybir.AluOpType.is_ge,
                        fill=0.0, base=0,
                        pattern=[[1, RB]], channel_multiplier=-1,
                    )
                    # fp8 path covers cols RB.. (rows RB..127 of q-tile 0
                    # ride the fp8 path; their n_keys >= RB keeps the fp8
                    # error averaged down)
                    chunks = _even_chunks(RB, s - RB, 512)
                else:
                    chunks = _even_chunks(k0, s - k0, 512)
                if t == n_s - 1 and kl < P:
                    # zero tail rows of partial tile for DoubleRow pairs
                    # (written first; valid rows are rewritten by exp below)
                    z0 = (kl // 32) * 32
                    nc.gpsimd.memset(expT8[z0:, t, k0:], 0.0)
                for pi, (c0, cl) in enumerate(chunks):
                    ext = min(256 - cl, c0) if cl < 256 else 0
                    sc = pp_score.tile([P, 512], F32, tag="score")
                    for j in range(n_a // 2):
                        nc.tensor.matmul(
                            sc[:kl, :ext + cl],
                            qkT8["k"][:, 2 * j:2 * j + 2, k0:k0 + kl],
                            qkT8["q"][:, 2 * j:2 * j + 2, c0 - ext:c0 + cl],
                            start=(j == 0), stop=(j == n_a // 2 - 1),
                            perf_mode=DR,
                        )
                    nc.scalar.activation(
                        expT8[:kl, t, c0:c0 + cl],
                        sc[:kl, ext:ext + cl], AF.Exp, scale=inv_den,
                    )
                    if pi == 0 and t > 0:
                        # causal mask on the diagonal block, post-exp
                        nc.gpsimd.affine_select(
                            out=expT8[:kl, t, k0:k0 + kl],
                            in_=expT8[:kl, t, k0:k0 + kl],
                            compare_op=mybir.AluOpType.is_ge,
                            fill=0.0, base=0,
                            pattern=[[1, kl]], channel_multiplier=-1,
                        )
                    if pi == 0 and t == 0:
                        # causal mask for cols RB..127 of the fp8 diag part
                        nc.gpsimd.affine_select(
                            out=expT8[:, 0, RB:P],
                            in_=expT8[:, 0, RB:P],
                            compare_op=mybir.AluOpType.is_ge,
                            fill=0.0, base=RB,
                            pattern=[[1, P - RB]], channel_multiplier=-1,
                        )
                # --- PV for q-tile i ---
                op1 = pp_o1.tile([P, h], F32, tag="op1")
                op2 = pp_o2.tile([P, a - h + 2], F32, tag="op2")
                if i == 0:
                    # rows 0..RB-1: accurate bf16 PV
                    nc.tensor.matmul(op1[:RB, :], e0b[:RB, :],
                                     v0b[:RB, 0:h], start=True, stop=True)
                    nc.tensor.matmul(op2[:RB, :], e0b[:RB, :],
                                     v0b[:RB, h:a + 2], start=True, stop=True)
                    # rows RB..127: fp8 PV over k-tile 0
                    nc.tensor.matmul(op1[RB:P, :], expT8[:, 0, RB:P],
                                     v8[:, 0, 0:h], start=True, stop=True)
                    nc.tensor.matmul(op2[RB:P, :], expT8[:, 0, RB:P],
                                     v8[:, 0, h:a + 2], start=True, stop=True)
                else:
                    npair = (i + 1) // 2
                    odd = (i + 1) % 2
                    for tp_ in range(npair):
                        lhs = expT8[:, 2 * tp_:2 * tp_ + 2, q0:q0 + il]
                        nc.tensor.matmul(
                            op1[:il, :], lhs,
                            v8[:, 2 * tp_:2 * tp_ + 2, 0:h],
                            start=(tp_ == 0),
                            stop=(tp_ == npair - 1 and not odd),
                            perf_mode=DR,
                        )
                        nc.tensor.matmul(
                            op2[:il, :], lhs,
                            v8[:, 2 * tp_:2 * tp_ + 2, h:a + 2],
                            start=(tp_ == 0),
                            stop=(tp_ == npair - 1 and not odd),
                            perf_mode=DR,
                        )
                    if odd:
                        lhs = expT8[:, i, q0:q0 + il]
                        nc.tensor.matmul(op1[:il, :], lhs, v8[:, i, 0:h],
                                         start=False, stop=True)
                        nc.tensor.matmul(op2[:il, :], lhs, v8[:, i, h:a + 2],
                                         start=False, stop=True)

                rec = pool.tile([P, 1], F32, tag="rec", bufs=2)
                nc.vector.reciprocal(rec[:il, :], op2[:il, a - h:a - h + 1])
                o_sb = pool.tile([P, a], F32, tag="o_sb", bufs=3)
                nc.vector.tensor_scalar_mul(
                    o_sb[:il, 0:h], op1[:il, :], rec[:il, 0:1])
                nc.vector.tensor_scalar_mul(
                    o_sb[:il, h:a], op2[:il, 0:a - h], rec[:il, 0:1])
                nc.sync.dma_start(out[b, q0:q0 + il, :], o_sb[:il, :])

    nc.compile()
    return nc


_BUILT = {}


def _get_nc(b_pc, s, e, a):
    key = (b_pc, s, e, a)
    if key not in _BUILT:
        _BUILT[key] = build(b_pc, s, e, a)
    return _BUILT[key]


def run_sharded(inputs, b_pc, s, e, a, **run_kwargs):
    """Run the SPMD kernel over N_CORES cores, sharding batch dim of x."""
    x = np.ascontiguousarray(inputs["x"], dtype=np.float32)
    b_total = x.shape[0]
    assert b_total == b_pc * N_CORES
    shared = {
        "Wq": np.ascontiguousarray(inputs["Wq"], dtype=np.float32),
        "Wk": np.ascontiguousarray(inputs["Wk"], dtype=np.float32),
        "Wv": np.ascontiguousarray(inputs["Wv"], dtype=np.float32),
        "bq": np.ascontiguousarray(inputs["bq"], dtype=np.float32),
        "bk": np.ascontiguousarray(inputs["bk"], dtype=np.float32),
        "bv": np.ascontiguousarray(inputs["bv"], dtype=np.float32),
    }
    in_maps = [
        {"x": x[c * b_pc:(c + 1) * b_pc], **shared} for c in range(N_CORES)
    ]
    nc = _get_nc(b_pc, s, e, a)
    res = run_bass_kernel_spmd(nc, in_maps, core_ids=list(range(N_CORES)),
                               **run_kwargs)
    full = np.concatenate([res.results[c]["out"] for c in range(N_CORES)], axis=0)
    return full, res


def kernel(**inputs) -> np.ndarray:
    out, _ = run_sharded(inputs, B // N_CORES, S, E, A)
    return out


# revision 21
# speedup vs baseline: 1.1988x; 1.1988x over previous
"""Masked self-attention Trainium2 Bass kernel (fp8 DoubleRow edition).

Reference computation (per batch b):
    q = x @ Wq + bq ; k = x @ Wk + bk ; v = x @ Wv + bv      # [S, A]
    scores = (q @ k.T) / sqrt(S)  with causal mask            # [S, S]
    out = softmax(scores, axis=-1) @ v                        # [S, A]

Sharding: data-parallel over batch across 8 NeuronCores (B=32 -> 4 per core),
weights replicated. No collectives.

Per-core design. The PE is the bottleneck (92% busy in the fp32r version),
so the big matmuls run in fp8e4 (e4m3) with MatmulPerfMode.DoubleRow: two
128-row contraction subtiles packed as [128, 2, free] APs, 2x the fp32r/bf16
rate. Error budget (gate 2e-2, fp32r baseline 2.4e-4): fp8 weights/acts give
~5% per-element error on q/k/v. That is fine wherever softmax averages many
keys, but output row q attends q+1 keys, so rows 0..127 (whose only key tile
is tile 0) need an accurate path. numpy emulation of exactly this split
measures 6.7e-3 max-rel error.

  fp8 main path (all of S, used by output rows >= 128):
    xT8 [E,S] fp8  <- PE-transpose of DMA-fed x (fp32r, 1.5cyc/row), DVE
                      PSUM->SBUF copy quantizes.
    qT8/kT8 [A,S] fp8 <- DoubleRow proj (W8 pairs x xT8 pairs), ACT copy
                      adds bias, quantizes. Unscaled (1/sqrt(S) folds into
                      the exp activation's scale to keep fp8 values in the
                      normal range).
    v8 [S,A+2] fp8 <- DoubleRow proj for s-tiles 1..7; tile 0 quantized from
                      the accurate bf16 tile-0 PSUM. Two ones-columns give
                      softmax row sums via the PV matmul.
    scoresT per k-tile = DoubleRow(kT8 pair, qT8 pair) -> fp32 PSUM; -1e9
                      additive mask on diagonal blocks; ACT exp(scale*x) ->
                      expT8 fp8. Tail rows of partial tile 7 are zeroed so
                      DoubleRow pairs never read garbage.
    PV per q-tile i>=1: DoubleRow over k-tile pairs (+ single fp8 matmul for
                      odd leftover tile), accumulating [il, 256]+[il, 258]
                      PSUM; sum column -> DVE reciprocal -> scale; GPSIMD
                      adds bv; DMA out per 256-col half.
  accurate tile-0 path (output rows 0..127):
    xT0b [E,128] bf16 (extra copy of the s-tile-0 transposes), bf16 weights.
    qT0b/kT0b [A,128] bf16 proj; diag scores bf16; exp -> e0b bf16;
    v0b [128,A+2] bf16; PV0 = (e0b, v0b) bf16.
"""

import numpy as np
from contextlib import ExitStack

import concourse.bass as bass
import concourse.mybir as mybir
import concourse.tile as tile
from concourse import bacc
from concourse.bass_utils import run_bass_kernel_spmd
from concourse.masks import make_identity

P = 128
F32 = mybir.dt.float32
F32R = mybir.dt.float32r
F8 = mybir.dt.float8e4
BF = mybir.dt.bfloat16
AF = mybir.ActivationFunctionType
DR = mybir.MatmulPerfMode.DoubleRow

N_CORES = 8
B, S, E, A = 32, 1000, 1024, 512
MASK_NEG = -1.0e9
RB = 64  # rows handled by the accurate bf16 path (error ~ 1/sqrt(n_keys))


def _even_chunks(start, total, maxc):
    """Split [start, start+total) into ceil(total/maxc) near-even chunks,
    each of even size."""
    assert total % 2 == 0
    n = max(1, -(-total // maxc))
    bounds = [start + ((i * total) // n) // 2 * 2 for i in range(n)]
    bounds.append(start + total)
    return [(bounds[i], bounds[i + 1] - bounds[i]) for i in range(n)]


def build(b_pc, s, e, a, reps=1):
    assert e % P == 0 and a % P == 0
    n_s = -(-s // P)
    n_e = e // P
    n_a = a // P
    inv_den = float(s) ** -0.5
    s_tiles = [(t * P, min(P, s - t * P)) for t in range(n_s)]
    h = a // 2  # PV column split: [0,h) and [h, a+2)
    # DoubleRow pair strides (dim1 of a [P, n, cols] tile) must be multiples
    # of 64 bytes, so pad the inner dim of every pair-sliced fp8 tile
    s_pad = -(-s // 64) * 64
    v_pad = -(-(a + 2) // 64) * 64

    nc = bacc.Bacc("TRN2")
    x = nc.dram_tensor("x", [b_pc, s, e], F32R, kind="ExternalInput").ap()
    w_dram = {
        "q": nc.dram_tensor("Wq", [e, a], F32, kind="ExternalInput").ap(),
        "k": nc.dram_tensor("Wk", [e, a], F32, kind="ExternalInput").ap(),
        "v": nc.dram_tensor("Wv", [e, a], F32, kind="ExternalInput").ap(),
    }
    b_dram = {
        "q": nc.dram_tensor("bq", [a], F32, kind="ExternalInput").ap(),
        "k": nc.dram_tensor("bk", [a], F32, kind="ExternalInput").ap(),
        "v": nc.dram_tensor("bv", [a], F32, kind="ExternalInput").ap(),
    }
    out = nc.dram_tensor("out", [b_pc, s, a], F32, kind="ExternalOutput").ap()

    with tile.TileContext(nc) as tc, ExitStack() as ctx:
        pool = ctx.enter_context(tc.tile_pool(name="sb", bufs=1))
        pp_tp = ctx.enter_context(tc.tile_pool(name="pp_tp", bufs=2, space="PSUM"))
        pp_proj = ctx.enter_context(tc.tile_pool(name="pp_proj", bufs=2, space="PSUM"))
        pp_score = ctx.enter_context(tc.tile_pool(name="pp_sc", bufs=2, space="PSUM"))
        pp_o1 = ctx.enter_context(tc.tile_pool(name="pp_o1", bufs=1, space="PSUM"))
        pp_o2 = ctx.enter_context(tc.tile_pool(name="pp_o2", bufs=1, space="PSUM"))

        # ---------------- constants ----------------
        ident_st = pool.tile([P, P], F32)
        make_identity(nc, ident_st)
        ident = pool.tile([P, P], F32R)
        nc.scalar.copy(ident[:], ident_st[:])

        # ---------------- weights / biases ----------------
        # w8: [P, n_e, a] fp8 (DoubleRow pair-sliceable); wb: bf16 (tile-0)
        w8, wb = {}, {}
        for nm in ("q", "k", "v"):
            w8[nm] = pool.tile([P, n_e, a], F8, tag=f"w8_{nm}", bufs=1,
                               name=f"w8_{nm}")
            wb[nm] = pool.tile([P, n_e, a], BF, tag=f"wb_{nm}", bufs=1,
                               name=f"wb_{nm}")
            for u in range(n_e):
                w_stage = pool.tile([P, a], F32, tag="w_stage", bufs=2)
                nc.gpsimd.dma_start(w_stage[:], w_dram[nm][u * P:(u + 1) * P, :])
                nc.vector.tensor_copy(w8[nm][:, u], w_stage[:])
                nc.vector.tensor_copy(wb[nm][:, u], w_stage[:])

        bias_sb = {}
        for nm in ("q", "k"):
            b_st = pool.tile([P, n_a], F32, tag=f"b_{nm}", bufs=1)
            nc.gpsimd.dma_start(
                b_st[:], b_dram[nm].rearrange("(m p) -> p m", p=P)
            )
            bias_sb[nm] = b_st

        bv_stage = pool.tile([1, a], F32)
        nc.gpsimd.dma_start(bv_stage[:], b_dram["v"][:])
        bv_bc = pool.tile([P, a], F32)
        nc.gpsimd.partition_broadcast(bv_bc[:], bv_stage[:])

        # ---------------- per-batch pipeline ----------------
        rep_ctx = (tc.For_i(0, reps, 1, hint_engines=tuple(nc.engines),
                            staggered_reset=True)
                   if reps > 1 else None)
        if rep_ctx is not None:
            ctx.enter_context(rep_ctx)
        for b in range(b_pc):
            # ---- tiles for this batch ----
            xT8 = pool.tile([P, n_e, s_pad], F8, tag="xT8", bufs=2,
                            name=f"xT8_{b}")
            xT0b = pool.tile([P, n_e, P], BF, tag="xT0b", bufs=2,
                             name=f"xT0b_{b}")
            qkT8 = {}
            for nm in ("q", "k"):
                qkT8[nm] = pool.tile([P, n_a, s_pad], F8, tag=f"{nm}T8",
                                     bufs=2, name=f"{nm}T8_{b}")
            qkT0b = {}
            for nm in ("q", "k"):
                qkT0b[nm] = pool.tile([P, n_a, RB], BF, tag=f"{nm}T0b",
                                      bufs=2, name=f"{nm}T0b_{b}")
            v8 = pool.tile([P, n_s, v_pad], F8, tag="v8", bufs=2,
                           name=f"v8_{b}")
            v0b = pool.tile([P, a + 2], BF, tag="v0b", bufs=2, name=f"v0b_{b}")
            g = 4 if n_e % 4 == 0 else (2 if n_e % 2 == 0 else 1)

            # ones columns (softmax row sums) + zeroed tail rows of partial
            # tile 7 (so DoubleRow pair reads see finite zeros, not junk);
            # valid rows/cols are overwritten by the PSUM copies below
            sl_last = s_tiles[-1][1]
            if sl_last < P:
                z0 = (sl_last // 32) * 32
                nc.gpsimd.memset(v8[z0:, n_s - 1, :a], 0.0)
            nc.gpsimd.memset(v8[:, :, a:a + 2], 1.0)
            nc.gpsimd.memset(v0b[:, a:a + 2], 1.0)

            def load_and_transpose(t):
                s0, sl = s_tiles[t]
                x_sb = pool.tile([P, e], F32R, tag="x", bufs=3, name="x_sb")
                # finer DMA split for batch 0, whose loads pace pipeline fill
                nsp = 4 if b == 0 else 2
                w_sp = e // nsp
                for qi in range(nsp):
                    nc.sync.dma_start(
                        x_sb[:sl, qi * w_sp:(qi + 1) * w_sp],
                        x[b, s0:s0 + sl, qi * w_sp:(qi + 1) * w_sp],
                    )
                for u0 in range(0, n_e, g):
                    tp = pp_tp.tile([P, g * P], F32R, tag="tp", name="tp")
                    for j in range(g):
                        nc.tensor.transpose(
                            tp[:, j * P:j * P + sl],
                            x_sb[:sl, (u0 + j) * P:(u0 + j + 1) * P],
                            ident[:sl, :sl],
                        )
                    tp3 = tp.rearrange("p (j c) -> p j c", c=P)
                    # alternate PSUM->SBUF copies across DVE/ACT to balance
                    if (t + u0 // g) % 2 == 0:
                        nc.vector.tensor_copy(
                            xT8[:, u0:u0 + g, s0:s0 + sl], tp3[:, :, :sl]
                        )
                    else:
                        nc.scalar.copy(
                            xT8[:, u0:u0 + g, s0:s0 + sl], tp3[:, :, :sl]
                        )
                    if s0 == 0:
                        nc.vector.tensor_copy(
                            xT0b[:, u0:u0 + g, :], tp3[:, :, :]
                        )

            def qk_chunk(nm, m, c0, cl):
                # fp8 qT/kT [A, S], unscaled, bias added in the ACT copy
                mm = pp_proj.tile([P, 512], F32, tag="proj", name="mm")
                for j in range(n_e // 2):
                    nc.tensor.matmul(
                        mm[:, :cl],
                        w8[nm][:, 2 * j:2 * j + 2, m * P:(m + 1) * P],
                        xT8[:, 2 * j:2 * j + 2, c0:c0 + cl],
                        start=(j == 0), stop=(j == n_e // 2 - 1),
                        perf_mode=DR,
                    )
                nc.scalar.activation(
                    qkT8[nm][:, m, c0:c0 + cl], mm[:, :cl], AF.Identity,
                    bias=bias_sb[nm][:, m:m + 1],
                )

            def v_tile(t):
                # fp8 DoubleRow proj; bv folded into v (softmax rows sum to
                # 1, so adding bv to every v row adds it to the output)
                s0, sl = s_tiles[t]
                vm = pp_proj.tile([P, 512], F32, tag="proj", name="vm")
                for j in range(n_e // 2):
                    nc.tensor.matmul(
                        vm[:sl, :a],
                        xT8[:, 2 * j:2 * j + 2, s0:s0 + sl],
                        w8["v"][:, 2 * j:2 * j + 2, :],
                        start=(j == 0), stop=(j == n_e // 2 - 1),
                        perf_mode=DR,
                    )
                nc.vector.tensor_add(v8[:sl, t, :a], vm[:sl, :a],
                                     bv_bc[:sl, :])

            # ---- stage A/B interleaved: first half of s, early proj ----
            for t in range(n_s // 2):
                load_and_transpose(t)

            # bf16 tile-0 qT/kT [A, RB] (accurate path for rows 0..RB-1)
            for nm in ("q", "k"):
                mm0 = pp_proj.tile([P, 512], F32, tag="proj", name="mm0")
                for m in range(n_a):
                    for u in range(n_e):
                        nc.tensor.matmul(
                            mm0[:, m * RB:(m + 1) * RB],
                            wb[nm][:, u, m * P:(m + 1) * P],
                            xT0b[:, u, :RB],
                            start=(u == 0), stop=(u == n_e - 1),
                        )
                for m in range(n_a):
                    nc.scalar.activation(
                        qkT0b[nm][:, m, :], mm0[:, m * RB:(m + 1) * RB],
                        AF.Identity, bias=bias_sb[nm][:, m:m + 1],
                    )

            # v tile 0: bf16 accurate proj -> both v0b and v8[:,0]
            vm0 = pp_proj.tile([P, 512], F32, tag="proj", name="vm0")
            for u in range(n_e):
                nc.tensor.matmul(
                    vm0[:, :a], xT0b[:, u, :], wb["v"][:, u, :],
                    start=(u == 0), stop=(u == n_e - 1),
                )
            nc.vector.tensor_add(v0b[:, :a], vm0[:, :a], bv_bc[:, :])
            nc.vector.tensor_add(v8[:, 0, :a], vm0[:, :a], bv_bc[:, :])

            half = (n_s // 2) * P  # 512: columns covered by tiles 0..3
            for nm in ("q", "k"):
                for m in range(n_a):
                    qk_chunk(nm, m, 0, half)
            for t in range(1, n_s // 2):
                v_tile(t)

            # ---- second half of s ----
            for t in range(n_s // 2, n_s):
                load_and_transpose(t)
            for nm in ("q", "k"):
                for m in range(n_a):
                    qk_chunk(nm, m, half, s - half)
            for t in range(n_s // 2, n_s):
                v_tile(t)

            # ---- stages C+D interleaved per tile ----
            expT8 = pool.tile([P, n_s, s_pad], F8, tag="expT8", bufs=2,
                              name=f"expT8_{b}")
            e0b = pool.tile([P, RB], BF, tag="e0b", bufs=2, name=f"e0b_{b}")

            for i, (q0, il) in enumerate(s_tiles):
                t, (k0, kl) = i, s_tiles[i]
                # --- scores + exp for k-tile t ---
                if t == 0:
                    # bf16 diagonal block (output rows 0..RB-1)
                    sc0 = pp_score.tile([P, 512], F32, tag="score")
                    for m in range(n_a):
                        nc.tensor.matmul(
                            sc0[:RB, :RB],
                            qkT0b["k"][:, m, :], qkT0b["q"][:, m, :],
                            start=(m == 0), stop=(m == n_a - 1),
                        )
                    nc.scalar.activation(e0b[:RB, :], sc0[:RB, :RB], AF.Exp,
                                         scale=inv_den)
                    # causal mask: zero exp where col q < row k (on GPSIMD,
                    # keeping DVE off the critical PSUM path; unmasked
                    # scores stay small enough that exp cannot overflow)
                    nc.gpsimd.affine_select(
                        out=e0b[:RB, :], in_=e0b[:RB, :],
                        compare_op=mybir.AluOpType.is_ge,
                        fill=0.0, base=0,
                        pattern=[[1, RB]], channel_multiplier=-1,
                    )
                    # fp8 path covers cols RB.. (rows RB..127 of q-tile 0
                    # ride the fp8 path; their n_keys >= RB keeps the fp8
                    # error averaged down)
                    chunks = _even_chunks(RB, s - RB, 512)
                else:
                    chunks = _even_chunks(k0, s - k0, 512)
                if t == n_s - 1 and kl < P:
                    # zero tail rows of partial tile for DoubleRow pairs
                    # (written first; valid rows are rewritten by exp below)
                    z0 = (kl // 32) * 32
                    nc.gpsimd.memset(expT8[z0:, t, k0:], 0.0)
                for pi, (c0, cl) in enumerate(chunks):
                    ext = min(256 - cl, c0) if cl < 256 else 0
                    sc = pp_score.tile([P, 512], F32, tag="score")
                    for j in range(n_a // 2):
                        nc.tensor.matmul(
                            sc[:kl, :ext + cl],
                            qkT8["k"][:, 2 * j:2 * j + 2, k0:k0 + kl],
                            qkT8["q"][:, 2 * j:2 * j + 2, c0 - ext:c0 + cl],
                            start=(j == 0), stop=(j == n_a // 2 - 1),
                            perf_mode=DR,
                        )
                    nc.scalar.activation(
                        expT8[:kl, t, c0:c0 + cl],
                        sc[:kl, ext:ext + cl], AF.Exp, scale=inv_den,
                    )
                    if pi == 0 and t > 0:
                        # causal mask on the diagonal block, post-exp
                        nc.gpsimd.affine_select(
                            out=expT8[:kl, t, k0:k0 + kl],
                            in_=expT8[:kl, t, k0:k0 + kl],
                            compare_op=mybir.AluOpType.is_ge,
                            fill=0.0, base=0,
                            pattern=[[1, kl]], channel_multiplier=-1,
                        )
                    if pi == 0 and t == 0:
                        # causal mask for cols RB..127 of the fp8 diag part
                        nc.gpsimd.affine_select(
                            out=expT8[:, 0, RB:P],
                            in_=expT8[:, 0, RB:P],
                            compare_op=mybir.AluOpType.is_ge,
                            fill=0.0, base=RB,
                            pattern=[[1, P - RB]], channel_multiplier=-1,
                        )
                # --- PV for q-tile i ---
                op1 = pp_o1.tile([P, h], F32, tag="op1")
                op2 = pp_o2.tile([P, a - h + 2], F32, tag="op2")
                if i == 0:
                    # rows 0..RB-1: accurate bf16 PV
                    nc.tensor.matmul(op1[:RB, :], e0b[:RB, :],
                                     v0b[:RB, 0:h], start=True, stop=True)
                    nc.tensor.matmul(op2[:RB, :], e0b[:RB, :],
                                     v0b[:RB, h:a + 2], start=True, stop=True)
                    # rows RB..127: fp8 PV over k-tile 0
                    nc.tensor.matmul(op1[RB:P, :], expT8[:, 0, RB:P],
                                     v8[:, 0, 0:h], start=True, stop=True)
                    nc.tensor.matmul(op2[RB:P, :], expT8[:, 0, RB:P],
                                     v8[:, 0, h:a + 2], start=True, stop=True)
                else:
                    npair = (i + 1) // 2
                    odd = (i + 1) % 2
                    for tp_ in range(npair):
                        lhs = expT8[:, 2 * tp_:2 * tp_ + 2, q0:q0 + il]
                        nc.tensor.matmul(
                            op1[:il, :], lhs,
                            v8[:, 2 * tp_:2 * tp_ + 2, 0:h],
                            start=(tp_ == 0),
                            stop=(tp_ == npair - 1 and not odd),
                            perf_mode=DR,
                        )
                        nc.tensor.matmul(
                            op2[:il, :], lhs,
                            v8[:, 2 * tp_:2 * tp_ + 2, h:a + 2],
                            start=(tp_ == 0),
                            stop=(tp_ == npair - 1 and not odd),
                            perf_mode=DR,
                        )
                    if odd:
                        lhs = expT8[:, i, q0:q0 + il]
                        nc.tensor.matmul(op1[:il, :], lhs, v8[:, i, 0:h],
                                         start=False, stop=True)
                        nc.tensor.matmul(op2[:il, :], lhs, v8[:, i, h:a + 2],
                                         start=False, stop=True)

                rec = pool.tile([P, 1], F32, tag="rec", bufs=2)
                nc.vector.reciprocal(rec[:il, :], op2[:il, a - h:a - h + 1])
                o_sb = pool.tile([P, a], F32, tag="o_sb", bufs=3)
                nc.vector.tensor_scalar_mul(
                    o_sb[:il, 0:h], op1[:il, :], rec[:il, 0:1])
                nc.vector.tensor_scalar_mul(
                    o_sb[:il, h:a], op2[:il, 0:a - h], rec[:il, 0:1])
                nc.sync.dma_start(out[b, q0:q0 + il, :], o_sb[:il, :])

    nc.compile()
    return nc


_BUILT = {}


def _get_nc(b_pc, s, e, a):
    key = (b_pc, s, e, a)
    if key not in _BUILT:
        _BUILT[key] = build(b_pc, s, e, a)
    return _BUILT[key]


def run_sharded(inputs, b_pc, s, e, a, **run_kwargs):
    """Run the SPMD kernel over N_CORES cores, sharding batch dim of x."""
    x = np.ascontiguousarray(inputs["x"], dtype=np.float32)
    b_total = x.shape[0]
    assert b_total == b_pc * N_CORES
    shared = {
        "Wq": np.ascontiguousarray(inputs["Wq"], dtype=np.float32),
        "Wk": np.ascontiguousarray(inputs["Wk"], dtype=np.float32),
        "Wv": np.ascontiguousarray(inputs["Wv"], dtype=np.float32),
        "bq": np.ascontiguousarray(inputs["bq"], dtype=np.float32),
        "bk": np.ascontiguousarray(inputs["bk"], dtype=np.float32),
        "bv": np.ascontiguousarray(inputs["bv"], dtype=np.float32),
    }
    in_maps = [
        {"x": x[c * b_pc:(c + 1) * b_pc], **shared} for c in range(N_CORES)
    ]
    nc = _get_nc(b_pc, s, e, a)
    res = run_bass_kernel_spmd(nc, in_maps, core_ids=list(range(N_CORES)),
                               **run_kwargs)
    full = np.concatenate([res.results[c]["out"] for c in range(N_CORES)], axis=0)
    return full, res


def kernel(**inputs) -> np.ndarray:
    out, _ = run_sharded(inputs, B // N_CORES, S, E, A)
    return out
